# revision 1
# baseline (speedup 1.0000x reference)
"""Trainium2 Bass kernel for nn_MeanAggregator (GNN mean aggregation).

out[b] = relu(concat(features[node[b]], mean_k features[neighbours[b,k]]) @ W)

8 NeuronCores, data-parallel over the batch (4096 items/core); features and W
replicated.  Host pre-concats node+neighbour indices into [B, 26] int32 and
pre-splits W into W_top and W_bot/25.

Per 128-item tile: 26 indirect DMAs (one per index column; the HW semantics
of indirect_dma_start take one index per partition) gather the node row and
the 25 neighbour rows into SBUF [128 items, 26*128].  TensorE sums the 25
neighbour slots via accumulated transpose-matmuls (is_transpose + start/stop
PSUM accumulation) into [dim, item] layout, transposes the node slot, then
two accumulated matmuls against W_top / (W_bot/25) produce the output; DVE
applies relu and the rows are stored.
"""

import sys

sys.path.insert(0, "/opt/trn_rl_repo")

import numpy as np

from concourse import bacc, bass, mybir, tile
from concourse.bass_utils import run_bass_kernel_spmd
from concourse.masks import make_identity

N_NODES = 100000
DIM = 128
B = 32768
K = 25
UNITS = 128
N_CORES = 8
P = 128
IDX_W = K + 1
GW = IDX_W * DIM


def build_program(n_items, n_nodes=N_NODES, dim=DIM, units=UNITS, k=K):
    idx_w = k + 1
    gw = idx_w * dim
    n_tiles = n_items // P
    assert n_items % P == 0

    nc = bacc.Bacc("TRN2", target_bir_lowering=False, debug=False)
    f32 = mybir.dt.float32
    feat = nc.dram_tensor("features", [n_nodes, dim], f32, kind="ExternalInput").ap()
    idx_all = nc.dram_tensor(
        "idx", [n_items, idx_w], mybir.dt.int32, kind="ExternalInput"
    ).ap()
    wt = nc.dram_tensor("wt", [dim, units], f32, kind="ExternalInput").ap()
    wb = nc.dram_tensor("wb", [dim, units], f32, kind="ExternalInput").ap()
    out = nc.dram_tensor("out", [n_items, units], f32, kind="ExternalOutput").ap()

    with tile.TileContext(nc) as tc:
        with (
            tc.tile_pool(name="const", bufs=1) as cpool,
            tc.tile_pool(name="gpool", bufs=3) as gpool,
            tc.tile_pool(name="spool", bufs=3) as spool,
            tc.tile_pool(name="opool", bufs=3) as opool,
            tc.tile_pool(name="pp", bufs=2, space="PSUM") as pp,
            tc.tile_pool(name="ppw", bufs=1, space="PSUM") as ppw,
        ):
            wt_sb = cpool.tile([dim, units], f32, tag="wt")
            nc.sync.dma_start(out=wt_sb[:], in_=wt[:])
            wb_sb = cpool.tile([dim, units], f32, tag="wb")
            nc.sync.dma_start(out=wb_sb[:], in_=wb[:])
            ident = cpool.tile([P, P], f32, tag="ident")
            make_identity(nc, ident[:])

            # idx_sb[p, t*idx_w + c] = idx_all[t*P + p, c]
            idx_sb = cpool.tile([P, n_tiles * idx_w], mybir.dt.int32, tag="idx")
            nc.sync.dma_start(
                out=idx_sb[:].rearrange("p (t c) -> p t c", c=idx_w),
                in_=idx_all.rearrange("(t p) c -> p t c", p=P),
            )

            # warmup matmuls absorb the constant-load waits
            psum_warm = ppw.tile([P, units], f32, tag="warm")
            nc.tensor.matmul(
                out=psum_warm[:], lhsT=ident[:], rhs=ident[:], is_transpose=True
            )
            nc.tensor.matmul(out=psum_warm[:], lhsT=ident[:], rhs=wt_sb[:])
            nc.tensor.matmul(out=psum_warm[:], lhsT=ident[:], rhs=wb_sb[:])

            for t in range(n_tiles):
                # 26 per-column gathers: slot c <- features[idx[t*P+p, c]]
                g = gpool.tile([P, gw], f32, tag="g")
                for c in range(idx_w):
                    nc.gpsimd.indirect_dma_start(
                        out=g[:, c * dim : (c + 1) * dim],
                        out_offset=None,
                        in_=feat[:],
                        in_offset=bass.IndirectOffsetOnAxis(
                            ap=idx_sb[:, t * idx_w + c : t * idx_w + c + 1],
                            axis=0,
                        ),
                    )

                psum_nb = pp.tile([P, P], f32, tag="nb")
                for j in range(k):
                    nc.tensor.matmul(
                        out=psum_nb[:],
                        lhsT=g[:, (1 + j) * dim : (2 + j) * dim],
                        rhs=ident[:],
                        is_transpose=True,
                        start=(j == 0),
                        stop=(j == k - 1),
                    )
                psum_nd = pp.tile([P, P], f32, tag="nd")
                nc.tensor.matmul(
                    out=psum_nd[:],
                    lhsT=g[:, 0:dim],
                    rhs=ident[:],
                    is_transpose=True,
                    start=True,
                    stop=True,
                )
                ndT = spool.tile([P, P], f32, tag="ndT")
                nc.vector.tensor_copy(out=ndT[:], in_=psum_nd[:])
                nbT = spool.tile([P, P], f32, tag="nbT")
                nc.vector.tensor_copy(out=nbT[:], in_=psum_nb[:])

                psum_o = pp.tile([P, units], f32, tag="o")
                nc.tensor.matmul(
                    out=psum_o[:], lhsT=ndT[:], rhs=wt_sb[:], start=True, stop=False
                )
                nc.tensor.matmul(
                    out=psum_o[:], lhsT=nbT[:], rhs=wb_sb[:], start=False, stop=True
                )
                o_sb = opool.tile([P, units], f32, tag="osb")
                nc.vector.tensor_relu(out=o_sb[:], in_=psum_o[:])
                nc.sync.dma_start(out=out[t * P : (t + 1) * P, :], in_=o_sb[:])

    nc.compile()
    return nc


_PROGRAM_CACHE = {}


def _get_program(n_items):
    if n_items not in _PROGRAM_CACHE:
        _PROGRAM_CACHE[n_items] = build_program(n_items)
    return _PROGRAM_CACHE[n_items]


def _prep_inputs(features, node, neighbours, W):
    features = np.ascontiguousarray(features, dtype=np.float32)
    node = np.asarray(node, dtype=np.int32).reshape(-1, 1)
    neighbours = np.asarray(neighbours, dtype=np.int32)
    W = np.asarray(W, dtype=np.float32)
    idx_all = np.ascontiguousarray(
        np.concatenate([node, neighbours], axis=1), dtype=np.int32
    )
    wt = np.ascontiguousarray(W[:DIM])
    wb = np.ascontiguousarray((W[DIM:].astype(np.float64) / K).astype(np.float32))
    return features, idx_all, wt, wb


def kernel(features, node, neighbours, W, trace=False):
    features, idx_all, wt, wb = _prep_inputs(features, node, neighbours, W)
    n_total = idx_all.shape[0]
    per_core = n_total // N_CORES
    nc = _get_program(per_core)
    in_maps = [
        {
            "features": features,
            "idx": idx_all[i * per_core : (i + 1) * per_core],
            "wt": wt,
            "wb": wb,
        }
        for i in range(N_CORES)
    ]
    res = run_bass_kernel_spmd(nc, in_maps, list(range(N_CORES)), trace=trace)
    out = np.concatenate([res.results[i]["out"] for i in range(N_CORES)], axis=0)
    if trace:
        kernel.last_result = res
    return out



# revision 9
# speedup vs baseline: 2.4117x; 2.4117x over previous
"""Trainium2 Bass kernel for nn_MeanAggregator (GNN mean aggregation).

out[b] = relu(concat(features[node[b]], mean_k features[neighbours[b,k]]) @ W)

8 NeuronCores, data-parallel over the batch (4096 items/core).  Tolerance is
2e-2, so features/W are cast to bf16 on the host (measured end-to-end error
~2.5e-3).

Gather strategy: `indirect_dma_start` costs ~1.1 us of Q7/SWDGE time per call
and can only gather 128 rows (one index per partition), so the 26 rows/item
gather would be Q7-bound.  Instead we use the vectorized `dma_gather`
(transpose=True): one instruction gathers 26624 rows (26 slots x 1024 items)
with ~0.34 ns/descriptor of Q7 time and lands them TRANSPOSED in SBUF as
[dim, slot-major columns] via the XBAR spray path.

dma_gather indices are int16 (15-bit usable), so the host builds, per
quarter-core (1024 items), a deduplicated sub-table of the ~23.4k unique
referenced rows (< 32767) and remaps indices into it.  The device then does
the full 26-rows/item expansion from HBM.

In transposed layout the whole aggregation collapses into matmuls: for each
512-item group, psum[u, b] += sum_d Wc[d, u] * gT[d, (c, b)] accumulated over
the 26 slots c (Wc = W_top for the node slot, W_bot/25 for neighbour slots)
— no PE transposes, no DVE tree, exact f32 PSUM accumulation.  ACT applies
relu; the output is produced transposed [units, items] and the host
transposes it back.
"""

import sys

sys.path.insert(0, "/opt/trn_rl_repo")

import numpy as np

from concourse import bacc, bass, mybir, tile
from concourse.bass_utils import run_bass_kernel_spmd

N_NODES = 100000
DIM = 128
B = 32768
K = 25
UNITS = 128
N_CORES = 8
P = 128
IDX_W = K + 1

Q_ITEMS = 1024  # items per gather batch (quarter of a core)
G_ITEMS = 512  # items per matmul group (one PSUM bank of f32)
N_GRP = Q_ITEMS // G_ITEMS
NI = IDX_W * Q_ITEMS  # indices per gather batch
IDX_COLS = NI // 16
U_MAX = 25000  # sub-table capacity (unique rows per batch ~23.4k +- 0.1k)

# one transposing dma_gather caps at 896 indices on real HW (1024 x 256 B
# hits a 2^18-byte limit in the descriptor path; HW-probed 896 ok / 1024
# wedges the device); chunk each quarter's gather accordingly
NI_CHUNK = 896
_bounds = list(range(0, NI, NI_CHUNK))
CHUNKS = [(a, min(a + NI_CHUNK, NI)) for a in _bounds]

BF16_NP = mybir.dt.np(mybir.dt.bfloat16)


def build_program(n_items):
    n_q = n_items // Q_ITEMS
    assert n_items % Q_ITEMS == 0

    nc = bacc.Bacc("TRN2", target_bir_lowering=False, debug=False,
                   num_swdge_queues=2)
    f32 = mybir.dt.float32
    bf16 = mybir.dt.bfloat16
    subtab = nc.dram_tensor(
        "subtab", [n_q, U_MAX, DIM], bf16, kind="ExternalInput"
    ).ap()
    idx16 = nc.dram_tensor(
        "idx16", [n_q, P, IDX_COLS], mybir.dt.int16, kind="ExternalInput"
    ).ap()
    wt = nc.dram_tensor("wt", [DIM, UNITS], bf16, kind="ExternalInput").ap()
    wb = nc.dram_tensor("wb", [DIM, UNITS], bf16, kind="ExternalInput").ap()
    outT = nc.dram_tensor("outT", [UNITS, n_items], f32, kind="ExternalOutput").ap()

    relu = mybir.ActivationFunctionType.Relu

    with tile.TileContext(nc) as tc:
        with (
            tc.tile_pool(name="const", bufs=1) as cpool,
            tc.tile_pool(name="gpool", bufs=2) as gpool,
            tc.tile_pool(name="opool", bufs=3) as opool,
            tc.tile_pool(name="pp", bufs=2, space="PSUM") as pp,
            tc.tile_pool(name="ppw", bufs=1, space="PSUM") as ppw,
        ):
            wt_sb = cpool.tile([DIM, UNITS], bf16, tag="wt")
            nc.sync.dma_start(out=wt_sb[:], in_=wt[:])
            wb_sb = cpool.tile([DIM, UNITS], bf16, tag="wb")
            nc.sync.dma_start(out=wb_sb[:], in_=wb[:])

            # idx_sb[p, q*IDX_COLS + c] = idx16[q, p, c]
            idx_sb = cpool.tile([P, n_q * IDX_COLS], mybir.dt.int16, tag="idx")
            nc.sync.dma_start(
                out=idx_sb[:].rearrange("p (q c) -> p q c", c=IDX_COLS),
                in_=idx16.rearrange("q p c -> p q c"),
            )

            # warmup matmuls absorb the constant-load waits
            psum_warm = ppw.tile([P, UNITS], f32, tag="warm")
            nc.tensor.matmul(out=psum_warm[:], lhsT=wt_sb[:], rhs=wt_sb[:])
            nc.tensor.matmul(out=psum_warm[:], lhsT=wb_sb[:], rhs=wb_sb[:])

            chunk_no = 0
            for q in range(n_q):
                # transposing gather of 1024 items x 26 slots, chunked to fit
                # the SWDGE ring: gT[d, (g*IDX_W + c)*G_ITEMS + b] =
                # subtab[q, idx[(g,c,b)], d]
                gT = gpool.tile([P, NI], bf16, tag="gT")
                for a, b in CHUNKS:
                    n = b - a
                    nc.gpsimd.dma_gather(
                        gT[:, a:b].rearrange("p (x n) -> p x n", x=1),
                        subtab[q],
                        idx_sb[:, q * IDX_COLS + a // 16 : q * IDX_COLS + b // 16],
                        n,
                        n,
                        DIM,
                        transpose=True,
                        queue_num=chunk_no % 2,
                    )
                    chunk_no += 1

                for g in range(N_GRP):
                    psum = pp.tile([P, G_ITEMS], f32, tag="ps")
                    for c in range(IDX_W):
                        col = (g * IDX_W + c) * G_ITEMS
                        nc.tensor.matmul(
                            out=psum[:],
                            lhsT=(wt_sb if c == 0 else wb_sb)[:],
                            rhs=gT[:, col : col + G_ITEMS],
                            start=(c == 0),
                            stop=(c == IDX_W - 1),
                        )
                    o_sb = opool.tile([P, G_ITEMS], f32, tag="osb")
                    nc.scalar.activation(out=o_sb[:], in_=psum[:], func=relu)
                    col = (q * N_GRP + g) * G_ITEMS
                    nc.sync.dma_start(
                        out=outT[:, col : col + G_ITEMS], in_=o_sb[:]
                    )

    nc.compile()
    return nc


_PROGRAM_CACHE = {}


def _get_program(n_items):
    if n_items not in _PROGRAM_CACHE:
        _PROGRAM_CACHE[n_items] = build_program(n_items)
    return _PROGRAM_CACHE[n_items]


def _prep_core(features_bf, idx_core):
    """Per-core host prep: dedup per quarter, build sub-tables + wrapped
    int16 slot-major indices."""
    n_q = idx_core.shape[0] // Q_ITEMS
    subtab = np.zeros((n_q, U_MAX, DIM), dtype=BF16_NP)
    idx16 = np.empty((n_q, P, IDX_COLS), dtype=np.int16)
    for q in range(n_q):
        sl = idx_core[q * Q_ITEMS : (q + 1) * Q_ITEMS]  # [Q_ITEMS, IDX_W]
        uniq, inv = np.unique(sl, return_inverse=True)
        assert len(uniq) <= U_MAX, f"unique rows {len(uniq)} > {U_MAX}"
        subtab[q, : len(uniq)] = features_bf[uniq]
        inv = inv.reshape(Q_ITEMS, IDX_W).astype(np.int16)
        # index order j = (g*IDX_W + c)*G_ITEMS + b
        ordered = np.empty(NI, dtype=np.int16)
        for g in range(N_GRP):
            blk = inv[g * G_ITEMS : (g + 1) * G_ITEMS]  # [G_ITEMS, IDX_W]
            ordered[g * IDX_W * G_ITEMS : (g + 1) * IDX_W * G_ITEMS] = (
                blk.T.ravel()
            )
        # wrap each gather chunk into 16 partitions (within-chunk
        # j = col*16 + p), replicate to 128
        wrapped = np.concatenate(
            [ordered[a:b].reshape(-1, 16).T for a, b in CHUNKS], axis=1
        )
        idx16[q] = np.tile(wrapped, (8, 1))
    return subtab, idx16


def _prep_inputs(features, node, neighbours, W):
    features_bf = np.asarray(features, dtype=np.float32).astype(BF16_NP)
    node = np.asarray(node, dtype=np.int32).reshape(-1, 1)
    neighbours = np.asarray(neighbours, dtype=np.int32)
    W = np.asarray(W, dtype=np.float32)
    idx_all = np.ascontiguousarray(
        np.concatenate([node, neighbours], axis=1), dtype=np.int32
    )
    wt = np.ascontiguousarray(W[:DIM]).astype(BF16_NP)
    wb = (W[DIM:].astype(np.float64) / K).astype(BF16_NP)
    return features_bf, idx_all, wt, wb


def kernel(features, node, neighbours, W, trace=False):
    features_bf, idx_all, wt, wb = _prep_inputs(features, node, neighbours, W)
    n_total = idx_all.shape[0]
    per_core = n_total // N_CORES
    nc = _get_program(per_core)
    in_maps = []
    for i in range(N_CORES):
        subtab, idx16 = _prep_core(
            features_bf, idx_all[i * per_core : (i + 1) * per_core]
        )
        in_maps.append({"subtab": subtab, "idx16": idx16, "wt": wt, "wb": wb})
    res = run_bass_kernel_spmd(nc, in_maps, list(range(N_CORES)), trace=trace)
    out = np.ascontiguousarray(
        np.concatenate([res.results[i]["outT"] for i in range(N_CORES)], axis=1).T
    )
    if trace:
        kernel.last_result = res
    return out
